# revision 16
# baseline (speedup 1.0000x reference)
"""Trainium2 kernel for MinibatchDiscrimination.

reference:
    M = einsum('ni,ibk->nbk', x, T)            # (256, 256, 16)
    l1[n,m,b] = sum_k |M[n,b,k] - M[m,b,k]|
    out[m,b]  = sum_n exp(-l1[n,m,b]) - 1      # (256, 256)
    return concat([x, out], axis=1)            # (256, 1280)

Sharding: tensor-parallel over the B_extra=256 feature dim -> 32 features
per core, no collectives. Each core computes out[:, shard] as [32, 256]
(batch on partitions), host transposes and concatenates with x.

Per-core dataflow:
  MT[(b,k), m] = M[m, b, k]   via PE matmul, (b,k) in 4 chunks of 128
  For each n, chunk c:
    Rp = relu(MT_c - MT_c[:, n])      one dual-op tensor_scalar (sub, max)
    Rm = relu(MTn_c - MTn_c[:, n])    MTn = -MT  => relu(M[n]-M[m])
    Rp + Rm = |M[m]-M[n]| elementwise; PE ones-selector matmul contracts
    the 128 (b,k) partitions -> l1[b', m] accumulated in PSUM over (c, +/-)
  E = exp(-l1) on ACT straight from PSUM; DVE accumulates E over n.
  out_dev[b', m] = sum_n E - 1, DMA'd out as [32, 256].
"""

import sys

sys.path.insert(0, "/opt/trn_rl_repo")

import os
import numpy as np
import ml_dtypes

ACT_FRAC = int(os.environ.get("MBD_ACT_FRAC", "20"))
ADD_PE = int(os.environ.get("MBD_ADD_PE", "60"))
R_BUFS = int(os.environ.get("MBD_R_BUFS", "16"))
E_BUFS = int(os.environ.get("MBD_E_BUFS", "6"))
PSL1_BUFS = int(os.environ.get("MBD_PSL1_BUFS", "5"))
PSMT_BUFS = int(os.environ.get("MBD_PSMT_BUFS", "2"))

N = 256
IN_FEATURES = 1024
B_EXTRA = 256
K = 16
N_CORES = 8
B_LOCAL = B_EXTRA // N_CORES          # 32 features per core
BK = B_LOCAL * K                      # 512 = (b_local, k) flattened
N_CHUNKS = BK // 128                  # 4 partition chunks of (b,k)
I_CHUNKS = IN_FEATURES // 128         # 8 contraction chunks

_COMPILED = None


def _apply_tile_drain_patch():
    """walrus in this container caps Drain (CTRL) instructions at one sem
    wait; Tile's end-of-kernel drain carries one wait per outstanding proc.
    Split the waits across a chain of drains."""
    from concourse import mybir, tile
    from concourse.vector_clock import ScopedClock

    def _drain_and_barrier(self, tick_clock, wait_clock):
        drain_inst = self.nc.sync.drain()
        wait_clock.add_sem_waits(
            drain_inst.ins, ScopedClock({None: tick_clock.global_clock})
        )
        si = drain_inst.ins.sync_info
        if si is not None and si.on_wait and len(si.on_wait) > 1:
            waits = list(si.on_wait)
            drain_inst.ins.sync_info = mybir.SyncInfo(
                on_wait=[waits[0]], on_update=list(si.on_update or [])
            )
            for w in waits[1:]:
                d = self.nc.sync.drain()
                d.ins.sync_info = mybir.SyncInfo(on_wait=[w], on_update=[])

        self.nc.all_engine_barrier()
        assert self.sems is not None
        popped = self.nc._tile_sem_poison_stack.pop()
        assert popped is self._sem_poison
        self.nc.clear_and_free_semaphores(list(self.sems.allocated().values()))
        self.nc.all_engine_barrier()

    tile.TileContext._drain_and_barrier = _drain_and_barrier


def _split_multi_waits(nc, max_waits=1):
    """This walrus build accepts at most one sync wait per instruction.
    Hoist extra waits onto NoOp instructions inserted just before the
    offending instruction in the same engine's stream."""
    from concourse import mybir

    cnt = 0
    for blk in nc.main_func.blocks:
        insts = blk.instructions
        if not any(
            inst.sync_info is not None
            and inst.sync_info.on_wait
            and len(inst.sync_info.on_wait) > max_waits
            for inst in insts
        ):
            continue
        new_list = []
        for inst in insts:
            si = inst.sync_info
            if si is not None and si.on_wait and len(si.on_wait) > max_waits:
                waits = list(si.on_wait)
                for w in waits[:-max_waits]:
                    nop = mybir.InstNoOp(name=f"wsplit-{cnt}", ins=[], outs=[])
                    cnt += 1
                    nop.engine = inst.engine
                    nop.sync_info = mybir.SyncInfo(on_wait=[w], on_update=[])
                    new_list.append(nop)
                inst.sync_info = mybir.SyncInfo(
                    on_wait=waits[-max_waits:],
                    on_update=list(si.on_update or []),
                )
            new_list.append(inst)
        insts[:] = new_list
    return cnt


def _build():
    from concourse import bass, mybir, tile

    _apply_tile_drain_patch()
    A = mybir.AluOpType
    F32 = mybir.dt.float32
    BF16 = mybir.dt.bfloat16

    nc = bass.Bass()
    xt_d = nc.declare_dram_parameter("xT", [IN_FEATURES, N], F32, isOutput=False)
    t_d = nc.declare_dram_parameter("Tsh", [IN_FEATURES, BK], F32, isOutput=False)
    w_d = nc.declare_dram_parameter("W", [128, N_CHUNKS * B_LOCAL], BF16,
                                    isOutput=False)
    w2_d = nc.declare_dram_parameter("W2", [128, N_CHUNKS * B_LOCAL], BF16,
                                    isOutput=False)
    wf_d = nc.declare_dram_parameter("Wf", [128, N_CHUNKS * B_LOCAL], F32,
                                    isOutput=False)
    i_d = nc.declare_dram_parameter("I32", [B_LOCAL, B_LOCAL], BF16,
                                    isOutput=False)
    if_d = nc.declare_dram_parameter("I32F", [B_LOCAL, B_LOCAL], F32,
                                     isOutput=False)
    out_d = nc.declare_dram_parameter("out", [B_LOCAL, N], F32, isOutput=True)

    with tile.TileContext(nc) as tc:
        with (
            tc.tile_pool(name="const", bufs=1) as const_pool,
            tc.tile_pool(name="mt", bufs=1) as mt_pool,
            tc.tile_pool(name="r", bufs=R_BUFS) as r_pool,
            tc.tile_pool(name="e", bufs=E_BUFS) as e_pool,
            tc.tile_pool(name="psmt", bufs=PSMT_BUFS, space="PSUM") as psmt_pool,
            tc.tile_pool(name="psl1", bufs=PSL1_BUFS, space="PSUM") as psl1_pool,
            tc.tile_pool(name="psacc", bufs=1, space="PSUM") as psacc_pool,
        ):
            # ---- load inputs ----
            xt = const_pool.tile([128, I_CHUNKS, N], F32, tag="xt")
            nc.sync.dma_start(
                xt[:], xt_d.rearrange("(c p) m -> p c m", p=128))
            tsh = const_pool.tile([128, I_CHUNKS, BK], F32, tag="tsh")
            nc.sync.dma_start(
                tsh[:], t_d.rearrange("(c p) m -> p c m", p=128))
            w_sb = const_pool.tile([128, N_CHUNKS * B_LOCAL], BF16, tag="w")
            nc.sync.dma_start(w_sb[:], w_d[:])
            w2_sb = const_pool.tile([128, N_CHUNKS * B_LOCAL], BF16, tag="w2")
            nc.sync.dma_start(w2_sb[:], w2_d[:])
            wf_sb = const_pool.tile([128, N_CHUNKS * B_LOCAL], F32, tag="wf")
            nc.sync.dma_start(wf_sb[:], wf_d[:])
            i_sb = const_pool.tile([B_LOCAL, B_LOCAL], BF16, tag="i32")
            nc.sync.dma_start(i_sb[:], i_d[:])
            if_sb = const_pool.tile([B_LOCAL, B_LOCAL], F32, tag="i32f")
            nc.sync.dma_start(if_sb[:], if_d[:])

            # ---- MT[(b,k), m] and -MT, per (b,k)-chunk ----
            # f32 copies feed the per-partition scalar operands; bf16
            # copies feed the streamed tensor_scalar input.
            mt_f, mtn_f, mt_b = [], [], []
            for c in range(N_CHUNKS):
                ps = psmt_pool.tile([128, N], F32)
                for ic in range(I_CHUNKS):
                    nc.tensor.matmul(
                        ps[:],
                        tsh[:, ic, 128 * c:128 * (c + 1)],
                        xt[:, ic, :],
                        start=(ic == 0),
                        stop=(ic == I_CHUNKS - 1),
                    )
                mf = mt_pool.tile([128, N], F32, tag=f"mtf{c}")
                nc.vector.tensor_copy(mf[:], ps[:])
                nf = mt_pool.tile([128, N], F32, tag=f"mtnf{c}")
                nc.vector.tensor_scalar(nf[:], ps[:], -1.0, None, A.mult)
                mb = mt_pool.tile([128, N], BF16, tag=f"mtb{c}")
                nc.vector.tensor_copy(mb[:], mf[:])
                mt_f.append(mf)
                mtn_f.append(nf)
                mt_b.append(mb)

            # ---- SS[b, m] = sum_k M[m, b, k] (f32), negST = -SS, and a
            # j-replicated copy for the per-group -SS[m] correction matmul.
            ss_ps = psmt_pool.tile([B_LOCAL, N], F32, tag="ps")
            for c in range(N_CHUNKS):
                nc.tensor.matmul(
                    ss_ps[:], wf_sb[:, B_LOCAL * c:B_LOCAL * (c + 1)],
                    mt_f[c][:], start=(c == 0), stop=(c == N_CHUNKS - 1))
            negst = mt_pool.tile([B_LOCAL, N], F32, tag="negst")
            nc.vector.tensor_scalar(negst[:], ss_ps[:], -1.0, None, A.mult)
            strep = mt_pool.tile([B_LOCAL, 16, N], F32, tag="strep")
            for j in range(16):
                nc.vector.tensor_copy(strep[:, j, :], negst[:])

            # ---- main loop: packs of n, upper triangle only ----
            # E[n, m] is symmetric. For a pack of P n's starting at even n0,
            # compute columns m in [n0, 256), width w = 256-n0. Pack size
            # grows as w shrinks so the selector matmul free dim stays near
            # 512. Row sums (m > n) accumulate into a PSUM-resident acc via
            # identity matmuls on PE; column sums (the transposed half) come
            # free from the exp's fused accum_out into accn[:, n].
            accn = e_pool.tile([B_LOCAL, N], F32, tag="accn")
            nc.gpsimd.memset(accn[:], 0.0)
            acc_sb = e_pool.tile([B_LOCAL, N], F32, tag="accsb")
            nc.gpsimd.memset(acc_sb[:], 0.0)
            acc_ps = psacc_pool.tile([B_LOCAL, N], F32)
            nc.vector.memset(acc_ps[:], 0.0)

            Exp = mybir.ActivationFunctionType.Exp
            Abs = mybir.ActivationFunctionType.Abs

            groups = []
            start = 0
            while start < N:
                w = N - start
                pack = 2 if w > 128 else (4 if w > 64 else (8 if w > 32 else 16))
                groups.append((start, pack, w))
                start += pack

            unit = 0
            addctr = 0
            for n0, pack, w in groups:
                ps = psl1_pool.tile([B_LOCAL, pack, w], F32)
                use_act = (unit * ACT_FRAC) % 100 < ACT_FRAC
                unit += 1
                if use_act:
                    # ACT path: |d| directly, plain selector, no correction
                    for c in range(N_CHUNKS):
                        wc = w_sb[:, B_LOCAL * c:B_LOCAL * (c + 1)]
                        r = r_pool.tile([128, pack, w], BF16, tag="r")
                        for j in range(pack):
                            nc.scalar.activation(
                                r[:, j, :], mt_b[c][:, n0:N], Abs,
                                bias=mtn_f[c][:, n0 + j:n0 + j + 1], scale=1.0)
                        nc.tensor.matmul(
                            ps[:], wc, r[:],
                            start=(c == 0), stop=(c == N_CHUNKS - 1))
                else:
                    # DVE path: l1 = 2*sum_k relu(d) - SS[m] + SS[n].
                    # The -SS[m] correction is the first matmul of the
                    # accumulation group; +SS[n] rides the exp bias.
                    nc.tensor.matmul(
                        ps[:], if_sb[:], strep[:, 0:pack, n0:N],
                        start=True, stop=False)
                    for c in range(N_CHUNKS):
                        wc = w2_sb[:, B_LOCAL * c:B_LOCAL * (c + 1)]
                        r = r_pool.tile([128, pack, w], BF16, tag="r")
                        for j in range(pack):
                            nc.vector.tensor_scalar(
                                r[:, j, :], mt_b[c][:, n0:N],
                                mt_f[c][:, n0 + j:n0 + j + 1], 0.0,
                                A.subtract, A.max)
                        nc.tensor.matmul(
                            ps[:], wc, r[:],
                            start=False, stop=(c == N_CHUNKS - 1))
                # exp only over the real terms of row j: cols [j+1, w)
                e = e_pool.tile([B_LOCAL, pack, w], BF16, tag="e")
                for j in range(pack):
                    n = n0 + j
                    if j + 1 >= w:
                        continue           # n = 255: no m > n terms
                    if use_act:
                        nc.scalar.activation(
                            e[:, j, j + 1:w], ps[:, j, j + 1:w], Exp,
                            bias=0.0, scale=-1.0,
                            accum_out=accn[:, n:n + 1])
                    else:
                        nc.scalar.activation(
                            e[:, j, j + 1:w], ps[:, j, j + 1:w], Exp,
                            bias=negst[:, n:n + 1], scale=-1.0,
                            accum_out=accn[:, n:n + 1])
                    # acc[:, n+1:256] += e
                    addctr += 1
                    if (addctr * ADD_PE) % 100 < ADD_PE:
                        nc.tensor.matmul(
                            acc_ps[:, n + 1:N], i_sb[:], e[:, j, j + 1:w],
                            start=False, stop=False,
                            skip_group_check=True)
                    else:
                        nc.vector.tensor_tensor(
                            acc_sb[:, n + 1:N], acc_sb[:, n + 1:N],
                            e[:, j, j + 1:w], A.add)

            # out[m, b]: col 0 has no row-half; combine acc halves + accn
            accf = e_pool.tile([B_LOCAL, N], F32, tag="accf")
            nc.vector.tensor_tensor(accf[:], accn[:], acc_sb[:], A.add)
            nc.vector.tensor_tensor(
                accf[:, 1:N], accf[:, 1:N], acc_ps[:, 1:N], A.add)
            nc.sync.dma_start(out_d[:], accf[:])

    _split_multi_waits(nc)
    return nc


def _selector_f32() -> np.ndarray:
    w = np.zeros((128, N_CHUNKS, B_LOCAL), dtype=np.float32)
    for c in range(N_CHUNKS):
        for p in range(128):
            w[p, c, (128 * c + p) // K] = 1.0
    return w.reshape(128, N_CHUNKS * B_LOCAL)


def kernel(x: np.ndarray, T: np.ndarray) -> np.ndarray:
    global _COMPILED
    from concourse.bass_utils import run_bass_kernel_spmd

    x = np.ascontiguousarray(x, dtype=np.float32)
    T = np.ascontiguousarray(T, dtype=np.float32)

    if _COMPILED is None:
        _COMPILED = _build()
    nc = _COMPILED

    xt = np.ascontiguousarray(x.T)                       # (1024, 256)
    wf = _selector_f32()
    w = wf.astype(ml_dtypes.bfloat16)
    w2 = (2.0 * wf).astype(ml_dtypes.bfloat16)
    eyef = np.eye(B_LOCAL, dtype=np.float32)
    eye = eyef.astype(ml_dtypes.bfloat16)
    in_maps = []
    for c in range(N_CORES):
        tsh = np.ascontiguousarray(
            T[:, c * B_LOCAL:(c + 1) * B_LOCAL, :].reshape(IN_FEATURES, BK))
        in_maps.append({"xT": xt, "Tsh": tsh, "W": w, "W2": w2, "Wf": wf,
                        "I32": eye, "I32F": eyef})

    res = run_bass_kernel_spmd(nc, in_maps, core_ids=list(range(N_CORES)))

    out = np.empty((N, IN_FEATURES + B_EXTRA), dtype=np.float32)
    out[:, :IN_FEATURES] = x
    for c in range(N_CORES):
        blk = res.results[c]["out"]                      # (32, 256) = (b, m)
        out[:, IN_FEATURES + c * B_LOCAL:IN_FEATURES + (c + 1) * B_LOCAL] = blk.T
    return out


# revision 17
# speedup vs baseline: 1.1240x; 1.1240x over previous
"""Trainium2 kernel for MinibatchDiscrimination.

reference:
    M = einsum('ni,ibk->nbk', x, T)            # (256, 256, 16)
    l1[n,m,b] = sum_k |M[n,b,k] - M[m,b,k]|
    out[m,b]  = sum_n exp(-l1[n,m,b]) - 1      # (256, 256)
    return concat([x, out], axis=1)            # (256, 1280)

Sharding: tensor-parallel over the B_extra=256 feature dim -> 32 features
per core, no collectives. Each core computes out[:, shard] as [32, 256]
(batch on partitions), host transposes and concatenates with x.

Per-core dataflow:
  MT[(b,k), m] = M[m, b, k]   via PE matmul, (b,k) in 4 chunks of 128
  For each n, chunk c:
    Rp = relu(MT_c - MT_c[:, n])      one dual-op tensor_scalar (sub, max)
    Rm = relu(MTn_c - MTn_c[:, n])    MTn = -MT  => relu(M[n]-M[m])
    Rp + Rm = |M[m]-M[n]| elementwise; PE ones-selector matmul contracts
    the 128 (b,k) partitions -> l1[b', m] accumulated in PSUM over (c, +/-)
  E = exp(-l1) on ACT straight from PSUM; DVE accumulates E over n.
  out_dev[b', m] = sum_n E - 1, DMA'd out as [32, 256].
"""

import sys

sys.path.insert(0, "/opt/trn_rl_repo")

import os
import numpy as np
import ml_dtypes

ACT_FRAC = int(os.environ.get("MBD_ACT_FRAC", "20"))
ADD_PE = int(os.environ.get("MBD_ADD_PE", "60"))
R_BUFS = int(os.environ.get("MBD_R_BUFS", "16"))
E_BUFS = int(os.environ.get("MBD_E_BUFS", "6"))
PSL1_BUFS = int(os.environ.get("MBD_PSL1_BUFS", "5"))
PSMT_BUFS = int(os.environ.get("MBD_PSMT_BUFS", "2"))

N = 256
IN_FEATURES = 1024
B_EXTRA = 256
K = 16
N_CORES = 8
B_LOCAL = B_EXTRA // N_CORES          # 32 features per core
BK = B_LOCAL * K                      # 512 = (b_local, k) flattened
N_CHUNKS = BK // 128                  # 4 partition chunks of (b,k)
I_CHUNKS = IN_FEATURES // 128         # 8 contraction chunks

_COMPILED = None


def _apply_tile_drain_patch():
    """walrus in this container caps Drain (CTRL) instructions at one sem
    wait; Tile's end-of-kernel drain carries one wait per outstanding proc.
    Split the waits across a chain of drains."""
    from concourse import mybir, tile
    from concourse.vector_clock import ScopedClock

    def _drain_and_barrier(self, tick_clock, wait_clock):
        drain_inst = self.nc.sync.drain()
        wait_clock.add_sem_waits(
            drain_inst.ins, ScopedClock({None: tick_clock.global_clock})
        )
        si = drain_inst.ins.sync_info
        if si is not None and si.on_wait and len(si.on_wait) > 1:
            waits = list(si.on_wait)
            drain_inst.ins.sync_info = mybir.SyncInfo(
                on_wait=[waits[0]], on_update=list(si.on_update or [])
            )
            for w in waits[1:]:
                d = self.nc.sync.drain()
                d.ins.sync_info = mybir.SyncInfo(on_wait=[w], on_update=[])

        self.nc.all_engine_barrier()
        assert self.sems is not None
        popped = self.nc._tile_sem_poison_stack.pop()
        assert popped is self._sem_poison
        self.nc.clear_and_free_semaphores(list(self.sems.allocated().values()))
        self.nc.all_engine_barrier()

    tile.TileContext._drain_and_barrier = _drain_and_barrier


def _split_multi_waits(nc, max_waits=1):
    """This walrus build accepts at most one sync wait per instruction.
    Hoist extra waits onto NoOp instructions inserted just before the
    offending instruction in the same engine's stream."""
    from concourse import mybir

    cnt = 0
    for blk in nc.main_func.blocks:
        insts = blk.instructions
        if not any(
            inst.sync_info is not None
            and inst.sync_info.on_wait
            and len(inst.sync_info.on_wait) > max_waits
            for inst in insts
        ):
            continue
        new_list = []
        for inst in insts:
            si = inst.sync_info
            if si is not None and si.on_wait and len(si.on_wait) > max_waits:
                waits = list(si.on_wait)
                for w in waits[:-max_waits]:
                    nop = mybir.InstNoOp(name=f"wsplit-{cnt}", ins=[], outs=[])
                    cnt += 1
                    nop.engine = inst.engine
                    nop.sync_info = mybir.SyncInfo(on_wait=[w], on_update=[])
                    new_list.append(nop)
                inst.sync_info = mybir.SyncInfo(
                    on_wait=waits[-max_waits:],
                    on_update=list(si.on_update or []),
                )
            new_list.append(inst)
        insts[:] = new_list
    return cnt


def _build():
    from concourse import bass, mybir, tile

    _apply_tile_drain_patch()
    A = mybir.AluOpType
    F32 = mybir.dt.float32
    BF16 = mybir.dt.bfloat16

    nc = bass.Bass()
    xt_d = nc.declare_dram_parameter("xT", [IN_FEATURES, N], F32, isOutput=False)
    t_d = nc.declare_dram_parameter("Tsh", [IN_FEATURES, BK], F32, isOutput=False)
    w_d = nc.declare_dram_parameter("W", [128, N_CHUNKS * B_LOCAL], BF16,
                                    isOutput=False)
    w2_d = nc.declare_dram_parameter("W2", [128, N_CHUNKS * B_LOCAL], BF16,
                                    isOutput=False)
    wf_d = nc.declare_dram_parameter("Wf", [128, N_CHUNKS * B_LOCAL], F32,
                                    isOutput=False)
    i_d = nc.declare_dram_parameter("I32", [B_LOCAL, B_LOCAL], BF16,
                                    isOutput=False)
    if_d = nc.declare_dram_parameter("I32F", [B_LOCAL, B_LOCAL], F32,
                                     isOutput=False)
    out_d = nc.declare_dram_parameter("out", [B_LOCAL, N], F32, isOutput=True)

    with tile.TileContext(nc) as tc:
        with (
            tc.tile_pool(name="const", bufs=1) as const_pool,
            tc.tile_pool(name="mt", bufs=1) as mt_pool,
            tc.tile_pool(name="r", bufs=R_BUFS) as r_pool,
            tc.tile_pool(name="e", bufs=E_BUFS) as e_pool,
            tc.tile_pool(name="psmt", bufs=PSMT_BUFS, space="PSUM") as psmt_pool,
            tc.tile_pool(name="psl1", bufs=PSL1_BUFS, space="PSUM") as psl1_pool,
            tc.tile_pool(name="psacc", bufs=1, space="PSUM") as psacc_pool,
        ):
            # ---- load inputs ----
            xt = const_pool.tile([128, I_CHUNKS, N], F32, tag="xt")
            nc.sync.dma_start(
                xt[:], xt_d.rearrange("(c p) m -> p c m", p=128))
            tsh = const_pool.tile([128, I_CHUNKS, BK], F32, tag="tsh")
            nc.sync.dma_start(
                tsh[:], t_d.rearrange("(c p) m -> p c m", p=128))
            w_sb = const_pool.tile([128, N_CHUNKS * B_LOCAL], BF16, tag="w")
            nc.sync.dma_start(w_sb[:], w_d[:])
            w2_sb = const_pool.tile([128, N_CHUNKS * B_LOCAL], BF16, tag="w2")
            nc.sync.dma_start(w2_sb[:], w2_d[:])
            wf_sb = const_pool.tile([128, N_CHUNKS * B_LOCAL], F32, tag="wf")
            nc.sync.dma_start(wf_sb[:], wf_d[:])
            i_sb = const_pool.tile([B_LOCAL, B_LOCAL], BF16, tag="i32")
            nc.sync.dma_start(i_sb[:], i_d[:])
            if_sb = const_pool.tile([B_LOCAL, B_LOCAL], F32, tag="i32f")
            nc.sync.dma_start(if_sb[:], if_d[:])

            # ---- MT[(b,k), m] and -MT, per (b,k)-chunk ----
            # f32 copies feed the per-partition scalar operands; bf16
            # copies feed the streamed tensor_scalar input.
            mt_f, mtn_f, mt_b = [], [], []
            for c in range(N_CHUNKS):
                ps = psmt_pool.tile([128, N], F32)
                for ic in range(I_CHUNKS):
                    nc.tensor.matmul(
                        ps[:],
                        tsh[:, ic, 128 * c:128 * (c + 1)],
                        xt[:, ic, :],
                        start=(ic == 0),
                        stop=(ic == I_CHUNKS - 1),
                    )
                mf = mt_pool.tile([128, N], F32, tag=f"mtf{c}")
                nc.vector.tensor_copy(mf[:], ps[:])
                nf = mt_pool.tile([128, N], F32, tag=f"mtnf{c}")
                nc.vector.tensor_scalar(nf[:], ps[:], -1.0, None, A.mult)
                mb = mt_pool.tile([128, N], BF16, tag=f"mtb{c}")
                nc.vector.tensor_copy(mb[:], mf[:])
                mt_f.append(mf)
                mtn_f.append(nf)
                mt_b.append(mb)

            # ---- SS[b, m] = sum_k M[m, b, k] (f32), negST = -SS, and a
            # j-replicated copy for the per-group -SS[m] correction matmul.
            ss_ps = psmt_pool.tile([B_LOCAL, N], F32, tag="ps")
            for c in range(N_CHUNKS):
                nc.tensor.matmul(
                    ss_ps[:], wf_sb[:, B_LOCAL * c:B_LOCAL * (c + 1)],
                    mt_f[c][:], start=(c == 0), stop=(c == N_CHUNKS - 1))
            negst = mt_pool.tile([B_LOCAL, N], F32, tag="negst")
            nc.vector.tensor_scalar(negst[:], ss_ps[:], -1.0, None, A.mult)
            # -SS as bf16 hi + bf16 residual lo: two cheap bf16 correction
            # matmuls instead of one slow f32 (hi/lo-pass) matmul.
            st_hi = mt_pool.tile([B_LOCAL, N], BF16, tag="sthi")
            nc.vector.tensor_copy(st_hi[:], negst[:])
            st_lo = mt_pool.tile([B_LOCAL, N], BF16, tag="stlo")
            nc.vector.tensor_tensor(st_lo[:], negst[:], st_hi[:], A.subtract)
            strep_h = mt_pool.tile([B_LOCAL, 16, N], BF16, tag="streph")
            strep_l = mt_pool.tile([B_LOCAL, 16, N], BF16, tag="strepl")
            for j in range(16):
                nc.vector.tensor_copy(strep_h[:, j, :], st_hi[:])
                nc.vector.tensor_copy(strep_l[:, j, :], st_lo[:])

            # ---- main loop: packs of n, upper triangle only ----
            # E[n, m] is symmetric. For a pack of P n's starting at even n0,
            # compute columns m in [n0, 256), width w = 256-n0. Pack size
            # grows as w shrinks so the selector matmul free dim stays near
            # 512. Row sums (m > n) accumulate into a PSUM-resident acc via
            # identity matmuls on PE; column sums (the transposed half) come
            # free from the exp's fused accum_out into accn[:, n].
            accn = e_pool.tile([B_LOCAL, N], F32, tag="accn")
            nc.gpsimd.memset(accn[:], 0.0)
            acc_sb = e_pool.tile([B_LOCAL, N], F32, tag="accsb")
            nc.gpsimd.memset(acc_sb[:], 0.0)
            acc_ps = psacc_pool.tile([B_LOCAL, N], F32)
            nc.vector.memset(acc_ps[:], 0.0)

            Exp = mybir.ActivationFunctionType.Exp
            Abs = mybir.ActivationFunctionType.Abs

            groups = []
            start = 0
            while start < N:
                w = N - start
                pack = 2 if w > 128 else (4 if w > 64 else (8 if w > 32 else 16))
                groups.append((start, pack, w))
                start += pack

            unit = 0
            addctr = 0
            for n0, pack, w in groups:
                ps = psl1_pool.tile([B_LOCAL, pack, w], F32)
                use_act = (unit * ACT_FRAC) % 100 < ACT_FRAC
                unit += 1
                if use_act:
                    # ACT path: |d| directly, plain selector, no correction
                    for c in range(N_CHUNKS):
                        wc = w_sb[:, B_LOCAL * c:B_LOCAL * (c + 1)]
                        r = r_pool.tile([128, pack, w], BF16, tag="r")
                        for j in range(pack):
                            nc.scalar.activation(
                                r[:, j, :], mt_b[c][:, n0:N], Abs,
                                bias=mtn_f[c][:, n0 + j:n0 + j + 1], scale=1.0)
                        nc.tensor.matmul(
                            ps[:], wc, r[:],
                            start=(c == 0), stop=(c == N_CHUNKS - 1))
                else:
                    # DVE path: l1 = 2*sum_k relu(d) - SS[m] + SS[n].
                    # The -SS[m] correction is the first matmul of the
                    # accumulation group; +SS[n] rides the exp bias.
                    nc.tensor.matmul(
                        ps[:], i_sb[:], strep_h[:, 0:pack, n0:N],
                        start=True, stop=False)
                    nc.tensor.matmul(
                        ps[:], i_sb[:], strep_l[:, 0:pack, n0:N],
                        start=False, stop=False)
                    for c in range(N_CHUNKS):
                        wc = w2_sb[:, B_LOCAL * c:B_LOCAL * (c + 1)]
                        r = r_pool.tile([128, pack, w], BF16, tag="r")
                        for j in range(pack):
                            nc.vector.tensor_scalar(
                                r[:, j, :], mt_b[c][:, n0:N],
                                mt_f[c][:, n0 + j:n0 + j + 1], 0.0,
                                A.subtract, A.max)
                        nc.tensor.matmul(
                            ps[:], wc, r[:],
                            start=False, stop=(c == N_CHUNKS - 1))
                # exp only over the real terms of row j: cols [j+1, w)
                e = e_pool.tile([B_LOCAL, pack, w], BF16, tag="e")
                for j in range(pack):
                    n = n0 + j
                    if j + 1 >= w:
                        continue           # n = 255: no m > n terms
                    if use_act:
                        nc.scalar.activation(
                            e[:, j, j + 1:w], ps[:, j, j + 1:w], Exp,
                            bias=0.0, scale=-1.0,
                            accum_out=accn[:, n:n + 1])
                    else:
                        nc.scalar.activation(
                            e[:, j, j + 1:w], ps[:, j, j + 1:w], Exp,
                            bias=negst[:, n:n + 1], scale=-1.0,
                            accum_out=accn[:, n:n + 1])
                    # acc[:, n+1:256] += e
                    addctr += 1
                    if (addctr * ADD_PE) % 100 < ADD_PE:
                        nc.tensor.matmul(
                            acc_ps[:, n + 1:N], i_sb[:], e[:, j, j + 1:w],
                            start=False, stop=False,
                            skip_group_check=True)
                    else:
                        nc.vector.tensor_tensor(
                            acc_sb[:, n + 1:N], acc_sb[:, n + 1:N],
                            e[:, j, j + 1:w], A.add)

            # out[m, b]: col 0 has no row-half; combine acc halves + accn
            accf = e_pool.tile([B_LOCAL, N], F32, tag="accf")
            nc.vector.tensor_tensor(accf[:], accn[:], acc_sb[:], A.add)
            nc.vector.tensor_tensor(
                accf[:, 1:N], accf[:, 1:N], acc_ps[:, 1:N], A.add)
            nc.sync.dma_start(out_d[:], accf[:])

    _split_multi_waits(nc)
    return nc


def _selector_f32() -> np.ndarray:
    w = np.zeros((128, N_CHUNKS, B_LOCAL), dtype=np.float32)
    for c in range(N_CHUNKS):
        for p in range(128):
            w[p, c, (128 * c + p) // K] = 1.0
    return w.reshape(128, N_CHUNKS * B_LOCAL)


def kernel(x: np.ndarray, T: np.ndarray) -> np.ndarray:
    global _COMPILED
    from concourse.bass_utils import run_bass_kernel_spmd

    x = np.ascontiguousarray(x, dtype=np.float32)
    T = np.ascontiguousarray(T, dtype=np.float32)

    if _COMPILED is None:
        _COMPILED = _build()
    nc = _COMPILED

    xt = np.ascontiguousarray(x.T)                       # (1024, 256)
    wf = _selector_f32()
    w = wf.astype(ml_dtypes.bfloat16)
    w2 = (2.0 * wf).astype(ml_dtypes.bfloat16)
    eyef = np.eye(B_LOCAL, dtype=np.float32)
    eye = eyef.astype(ml_dtypes.bfloat16)
    in_maps = []
    for c in range(N_CORES):
        tsh = np.ascontiguousarray(
            T[:, c * B_LOCAL:(c + 1) * B_LOCAL, :].reshape(IN_FEATURES, BK))
        in_maps.append({"xT": xt, "Tsh": tsh, "W": w, "W2": w2, "Wf": wf,
                        "I32": eye, "I32F": eyef})

    res = run_bass_kernel_spmd(nc, in_maps, core_ids=list(range(N_CORES)))

    out = np.empty((N, IN_FEATURES + B_EXTRA), dtype=np.float32)
    out[:, :IN_FEATURES] = x
    for c in range(N_CORES):
        blk = res.results[c]["out"]                      # (32, 256) = (b, m)
        out[:, IN_FEATURES + c * B_LOCAL:IN_FEATURES + (c + 1) * B_LOCAL] = blk.T
    return out


# revision 18
# speedup vs baseline: 1.3705x; 1.2193x over previous
"""Trainium2 kernel for MinibatchDiscrimination.

reference:
    M = einsum('ni,ibk->nbk', x, T)            # (256, 256, 16)
    l1[n,m,b] = sum_k |M[n,b,k] - M[m,b,k]|
    out[m,b]  = sum_n exp(-l1[n,m,b]) - 1      # (256, 256)
    return concat([x, out], axis=1)            # (256, 1280)

Sharding: tensor-parallel over the B_extra=256 feature dim -> 32 features
per core, no collectives. Each core computes out[:, shard] as [32, 256]
(batch on partitions), host transposes and concatenates with x.

Per-core dataflow:
  MT[(b,k), m] = M[m, b, k]   via PE matmul, (b,k) in 4 chunks of 128
  For each n, chunk c:
    Rp = relu(MT_c - MT_c[:, n])      one dual-op tensor_scalar (sub, max)
    Rm = relu(MTn_c - MTn_c[:, n])    MTn = -MT  => relu(M[n]-M[m])
    Rp + Rm = |M[m]-M[n]| elementwise; PE ones-selector matmul contracts
    the 128 (b,k) partitions -> l1[b', m] accumulated in PSUM over (c, +/-)
  E = exp(-l1) on ACT straight from PSUM; DVE accumulates E over n.
  out_dev[b', m] = sum_n E - 1, DMA'd out as [32, 256].
"""

import sys

sys.path.insert(0, "/opt/trn_rl_repo")

import os
import numpy as np
import ml_dtypes

ACT_FRAC = int(os.environ.get("MBD_ACT_FRAC", "10"))
ADD_PE = int(os.environ.get("MBD_ADD_PE", "100"))
R_BUFS = int(os.environ.get("MBD_R_BUFS", "16"))
E_BUFS = int(os.environ.get("MBD_E_BUFS", "6"))
PSL1_BUFS = int(os.environ.get("MBD_PSL1_BUFS", "5"))
PSMT_BUFS = int(os.environ.get("MBD_PSMT_BUFS", "2"))

N = 256
IN_FEATURES = 1024
B_EXTRA = 256
K = 16
N_CORES = 8
B_LOCAL = B_EXTRA // N_CORES          # 32 features per core
BK = B_LOCAL * K                      # 512 = (b_local, k) flattened
N_CHUNKS = BK // 128                  # 4 partition chunks of (b,k)
I_CHUNKS = IN_FEATURES // 128         # 8 contraction chunks

_COMPILED = None


def _apply_tile_drain_patch():
    """walrus in this container caps Drain (CTRL) instructions at one sem
    wait; Tile's end-of-kernel drain carries one wait per outstanding proc.
    Split the waits across a chain of drains."""
    from concourse import mybir, tile
    from concourse.vector_clock import ScopedClock

    def _drain_and_barrier(self, tick_clock, wait_clock):
        drain_inst = self.nc.sync.drain()
        wait_clock.add_sem_waits(
            drain_inst.ins, ScopedClock({None: tick_clock.global_clock})
        )
        si = drain_inst.ins.sync_info
        if si is not None and si.on_wait and len(si.on_wait) > 1:
            waits = list(si.on_wait)
            drain_inst.ins.sync_info = mybir.SyncInfo(
                on_wait=[waits[0]], on_update=list(si.on_update or [])
            )
            for w in waits[1:]:
                d = self.nc.sync.drain()
                d.ins.sync_info = mybir.SyncInfo(on_wait=[w], on_update=[])

        self.nc.all_engine_barrier()
        assert self.sems is not None
        popped = self.nc._tile_sem_poison_stack.pop()
        assert popped is self._sem_poison
        self.nc.clear_and_free_semaphores(list(self.sems.allocated().values()))
        self.nc.all_engine_barrier()

    tile.TileContext._drain_and_barrier = _drain_and_barrier


def _split_multi_waits(nc, max_waits=1):
    """This walrus build accepts at most one sync wait per instruction.
    Hoist extra waits onto NoOp instructions inserted just before the
    offending instruction in the same engine's stream."""
    from concourse import mybir

    cnt = 0
    for blk in nc.main_func.blocks:
        insts = blk.instructions
        if not any(
            inst.sync_info is not None
            and inst.sync_info.on_wait
            and len(inst.sync_info.on_wait) > max_waits
            for inst in insts
        ):
            continue
        new_list = []
        for inst in insts:
            si = inst.sync_info
            if si is not None and si.on_wait and len(si.on_wait) > max_waits:
                waits = list(si.on_wait)
                for w in waits[:-max_waits]:
                    nop = mybir.InstNoOp(name=f"wsplit-{cnt}", ins=[], outs=[])
                    cnt += 1
                    nop.engine = inst.engine
                    nop.sync_info = mybir.SyncInfo(on_wait=[w], on_update=[])
                    new_list.append(nop)
                inst.sync_info = mybir.SyncInfo(
                    on_wait=waits[-max_waits:],
                    on_update=list(si.on_update or []),
                )
            new_list.append(inst)
        insts[:] = new_list
    return cnt


def _build():
    from concourse import bass, mybir, tile

    _apply_tile_drain_patch()
    A = mybir.AluOpType
    F32 = mybir.dt.float32
    BF16 = mybir.dt.bfloat16

    nc = bass.Bass()
    xt_d = nc.declare_dram_parameter("xT", [IN_FEATURES, N], F32, isOutput=False)
    t_d = nc.declare_dram_parameter("Tsh", [IN_FEATURES, BK], F32, isOutput=False)
    w_d = nc.declare_dram_parameter("W", [128, N_CHUNKS * B_LOCAL], BF16,
                                    isOutput=False)
    w2_d = nc.declare_dram_parameter("W2", [128, N_CHUNKS * B_LOCAL], BF16,
                                    isOutput=False)
    wf_d = nc.declare_dram_parameter("Wf", [128, N_CHUNKS * B_LOCAL], F32,
                                    isOutput=False)
    i_d = nc.declare_dram_parameter("I32", [B_LOCAL, B_LOCAL], BF16,
                                    isOutput=False)
    if_d = nc.declare_dram_parameter("I32F", [B_LOCAL, B_LOCAL], F32,
                                     isOutput=False)
    out_d = nc.declare_dram_parameter("out", [B_LOCAL, N], F32, isOutput=True)

    with tile.TileContext(nc) as tc:
        with (
            tc.tile_pool(name="const", bufs=1) as const_pool,
            tc.tile_pool(name="mt", bufs=1) as mt_pool,
            tc.tile_pool(name="r", bufs=R_BUFS) as r_pool,
            tc.tile_pool(name="e", bufs=E_BUFS) as e_pool,
            tc.tile_pool(name="psmt", bufs=PSMT_BUFS, space="PSUM") as psmt_pool,
            tc.tile_pool(name="psl1", bufs=PSL1_BUFS, space="PSUM") as psl1_pool,
            tc.tile_pool(name="psacc", bufs=1, space="PSUM") as psacc_pool,
        ):
            # ---- load inputs ----
            xt = const_pool.tile([128, I_CHUNKS, N], F32, tag="xt")
            nc.sync.dma_start(
                xt[:], xt_d.rearrange("(c p) m -> p c m", p=128))
            tsh = const_pool.tile([128, I_CHUNKS, BK], F32, tag="tsh")
            nc.sync.dma_start(
                tsh[:], t_d.rearrange("(c p) m -> p c m", p=128))
            w_sb = const_pool.tile([128, N_CHUNKS * B_LOCAL], BF16, tag="w")
            nc.sync.dma_start(w_sb[:], w_d[:])
            w2_sb = const_pool.tile([128, N_CHUNKS * B_LOCAL], BF16, tag="w2")
            nc.sync.dma_start(w2_sb[:], w2_d[:])
            wf_sb = const_pool.tile([128, N_CHUNKS * B_LOCAL], F32, tag="wf")
            nc.sync.dma_start(wf_sb[:], wf_d[:])
            i_sb = const_pool.tile([B_LOCAL, B_LOCAL], BF16, tag="i32")
            nc.sync.dma_start(i_sb[:], i_d[:])
            if_sb = const_pool.tile([B_LOCAL, B_LOCAL], F32, tag="i32f")
            nc.sync.dma_start(if_sb[:], if_d[:])

            # ---- MT[(b,k), m] and -MT, per (b,k)-chunk ----
            # f32 copies feed the per-partition scalar operands; bf16
            # copies feed the streamed tensor_scalar input.
            mt_f, mtn_f, mt_b = [], [], []
            for c in range(N_CHUNKS):
                ps = psmt_pool.tile([128, N], F32)
                for ic in range(I_CHUNKS):
                    nc.tensor.matmul(
                        ps[:],
                        tsh[:, ic, 128 * c:128 * (c + 1)],
                        xt[:, ic, :],
                        start=(ic == 0),
                        stop=(ic == I_CHUNKS - 1),
                    )
                mf = mt_pool.tile([128, N], F32, tag=f"mtf{c}")
                nc.vector.tensor_copy(mf[:], ps[:])
                nf = mt_pool.tile([128, N], F32, tag=f"mtnf{c}")
                nc.vector.tensor_scalar(nf[:], ps[:], -1.0, None, A.mult)
                mb = mt_pool.tile([128, N], BF16, tag=f"mtb{c}")
                nc.vector.tensor_copy(mb[:], mf[:])
                mt_f.append(mf)
                mtn_f.append(nf)
                mt_b.append(mb)

            # ---- SS[b, m] = sum_k M[m, b, k] (f32), negST = -SS, and a
            # j-replicated copy for the per-group -SS[m] correction matmul.
            ss_ps = psmt_pool.tile([B_LOCAL, N], F32, tag="ps")
            for c in range(N_CHUNKS):
                nc.tensor.matmul(
                    ss_ps[:], wf_sb[:, B_LOCAL * c:B_LOCAL * (c + 1)],
                    mt_f[c][:], start=(c == 0), stop=(c == N_CHUNKS - 1))
            negst = mt_pool.tile([B_LOCAL, N], F32, tag="negst")
            nc.vector.tensor_scalar(negst[:], ss_ps[:], -1.0, None, A.mult)
            # -SS as bf16 hi + bf16 residual lo: two cheap bf16 correction
            # matmuls instead of one slow f32 (hi/lo-pass) matmul.
            st_hi = mt_pool.tile([B_LOCAL, N], BF16, tag="sthi")
            nc.vector.tensor_copy(st_hi[:], negst[:])
            st_lo = mt_pool.tile([B_LOCAL, N], BF16, tag="stlo")
            nc.vector.tensor_tensor(st_lo[:], negst[:], st_hi[:], A.subtract)
            strep_h = mt_pool.tile([B_LOCAL, 16, N], BF16, tag="streph")
            strep_l = mt_pool.tile([B_LOCAL, 16, N], BF16, tag="strepl")
            for j in range(16):
                nc.vector.tensor_copy(strep_h[:, j, :], st_hi[:])
                nc.vector.tensor_copy(strep_l[:, j, :], st_lo[:])

            # ---- main loop: packs of n, upper triangle only ----
            # E[n, m] is symmetric. For a pack of P n's starting at even n0,
            # compute columns m in [n0, 256), width w = 256-n0. Pack size
            # grows as w shrinks so the selector matmul free dim stays near
            # 512. Row sums (m > n) accumulate into a PSUM-resident acc via
            # identity matmuls on PE; column sums (the transposed half) come
            # free from the exp's fused accum_out into accn[:, n].
            accn = e_pool.tile([B_LOCAL, N], F32, tag="accn")
            nc.gpsimd.memset(accn[:], 0.0)
            acc_sb = e_pool.tile([B_LOCAL, N], F32, tag="accsb")
            nc.gpsimd.memset(acc_sb[:], 0.0)
            acc_ps = psacc_pool.tile([B_LOCAL, N], F32)
            nc.vector.memset(acc_ps[:], 0.0)

            Exp = mybir.ActivationFunctionType.Exp
            Abs = mybir.ActivationFunctionType.Abs

            groups = []
            start = 0
            while start < N:
                w = N - start
                pack = 2 if w > 128 else (4 if w > 64 else (8 if w > 32 else 16))
                groups.append((start, pack, w))
                start += pack

            unit = 0
            addctr = 0
            for n0, pack, w in groups:
                ps = psl1_pool.tile([B_LOCAL, pack, w], F32)
                use_act = (unit * ACT_FRAC) % 100 < ACT_FRAC
                unit += 1
                if use_act:
                    # ACT path: |d| directly, plain selector, no correction
                    for c in range(N_CHUNKS):
                        wc = w_sb[:, B_LOCAL * c:B_LOCAL * (c + 1)]
                        r = r_pool.tile([128, pack, w], BF16, tag="r")
                        for j in range(pack):
                            nc.scalar.activation(
                                r[:, j, :], mt_b[c][:, n0:N], Abs,
                                bias=mtn_f[c][:, n0 + j:n0 + j + 1], scale=1.0)
                        nc.tensor.matmul(
                            ps[:], wc, r[:],
                            start=(c == 0), stop=(c == N_CHUNKS - 1))
                else:
                    # DVE path: l1 = 2*sum_k relu(d) - SS[m] + SS[n].
                    # The -SS[m] correction is the first matmul of the
                    # accumulation group; +SS[n] rides the exp bias.
                    nc.tensor.matmul(
                        ps[:], i_sb[:], strep_h[:, 0:pack, n0:N],
                        start=True, stop=False)
                    nc.tensor.matmul(
                        ps[:], i_sb[:], strep_l[:, 0:pack, n0:N],
                        start=False, stop=False)
                    for c in range(N_CHUNKS):
                        wc = w2_sb[:, B_LOCAL * c:B_LOCAL * (c + 1)]
                        r = r_pool.tile([128, pack, w], BF16, tag="r")
                        for j in range(pack):
                            nc.vector.tensor_scalar(
                                r[:, j, :], mt_b[c][:, n0:N],
                                mt_f[c][:, n0 + j:n0 + j + 1], 0.0,
                                A.subtract, A.max)
                        nc.tensor.matmul(
                            ps[:], wc, r[:],
                            start=False, stop=(c == N_CHUNKS - 1))
                # exp only over the real terms of row j: cols [j+1, w)
                e = e_pool.tile([B_LOCAL, pack, w], BF16, tag="e")
                for j in range(pack):
                    n = n0 + j
                    if j + 1 >= w:
                        continue           # n = 255: no m > n terms
                    if use_act:
                        nc.scalar.activation(
                            e[:, j, j + 1:w], ps[:, j, j + 1:w], Exp,
                            bias=0.0, scale=-1.0,
                            accum_out=accn[:, n:n + 1])
                    else:
                        nc.scalar.activation(
                            e[:, j, j + 1:w], ps[:, j, j + 1:w], Exp,
                            bias=negst[:, n:n + 1], scale=-1.0,
                            accum_out=accn[:, n:n + 1])
                    # acc[:, n+1:256] += e
                    addctr += 1
                    if (addctr * ADD_PE) % 100 < ADD_PE:
                        nc.tensor.matmul(
                            acc_ps[:, n + 1:N], i_sb[:], e[:, j, j + 1:w],
                            start=False, stop=False,
                            skip_group_check=True)
                    else:
                        nc.vector.tensor_tensor(
                            acc_sb[:, n + 1:N], acc_sb[:, n + 1:N],
                            e[:, j, j + 1:w], A.add)

            # out[m, b]: col 0 has no row-half; combine acc halves + accn
            accf = e_pool.tile([B_LOCAL, N], F32, tag="accf")
            nc.vector.tensor_tensor(accf[:], accn[:], acc_sb[:], A.add)
            nc.vector.tensor_tensor(
                accf[:, 1:N], accf[:, 1:N], acc_ps[:, 1:N], A.add)
            nc.sync.dma_start(out_d[:], accf[:])

    _split_multi_waits(nc)
    return nc


def _selector_f32() -> np.ndarray:
    w = np.zeros((128, N_CHUNKS, B_LOCAL), dtype=np.float32)
    for c in range(N_CHUNKS):
        for p in range(128):
            w[p, c, (128 * c + p) // K] = 1.0
    return w.reshape(128, N_CHUNKS * B_LOCAL)


def kernel(x: np.ndarray, T: np.ndarray) -> np.ndarray:
    global _COMPILED
    from concourse.bass_utils import run_bass_kernel_spmd

    x = np.ascontiguousarray(x, dtype=np.float32)
    T = np.ascontiguousarray(T, dtype=np.float32)

    if _COMPILED is None:
        _COMPILED = _build()
    nc = _COMPILED

    xt = np.ascontiguousarray(x.T)                       # (1024, 256)
    wf = _selector_f32()
    w = wf.astype(ml_dtypes.bfloat16)
    w2 = (2.0 * wf).astype(ml_dtypes.bfloat16)
    eyef = np.eye(B_LOCAL, dtype=np.float32)
    eye = eyef.astype(ml_dtypes.bfloat16)
    in_maps = []
    for c in range(N_CORES):
        tsh = np.ascontiguousarray(
            T[:, c * B_LOCAL:(c + 1) * B_LOCAL, :].reshape(IN_FEATURES, BK))
        in_maps.append({"xT": xt, "Tsh": tsh, "W": w, "W2": w2, "Wf": wf,
                        "I32": eye, "I32F": eyef})

    res = run_bass_kernel_spmd(nc, in_maps, core_ids=list(range(N_CORES)))

    out = np.empty((N, IN_FEATURES + B_EXTRA), dtype=np.float32)
    out[:, :IN_FEATURES] = x
    for c in range(N_CORES):
        blk = res.results[c]["out"]                      # (32, 256) = (b, m)
        out[:, IN_FEATURES + c * B_LOCAL:IN_FEATURES + (c + 1) * B_LOCAL] = blk.T
    return out
